# revision 28
# baseline (speedup 1.0000x reference)
"""Trainium2 Bass kernel for nn_Attention_72559177499201.

Reference (per batch b):
  T = q_bar[b] @ Wg + bg                  (S, H)
  scores = T @ a_bar[b].T                 (S_q, S_a)
  g = softmax(scores, axis=q)             (softmax over the QUERY axis)
  h[b] = g.T-contracted with a_bar[b]:  h[a, :] = sum_q g[q, a] * a_bar[b, q, :]

Sharding: data-parallel over batch: B=16 across 8 cores, 2 batches/core.
Forward only -> no collectives.

Per-core plan (per batch):
  stage1: T^T[k, q] = sum_h Wg[h, k] * qT[h, q]   (f32r matmuls; qT via PE
          transpose; two bank-aligned PSUM rounds because start=True clears
          has_written for a whole bank)
  stage2: S_T[a, q] = aT_chunk^T @ T^T   (f32r; a-tile of 128 keys on
          partitions so the softmax axis q lands on the free axis)
  softmax along free axis of S_T: per-bank maxes + combine (DVE), one
          exp with bias=-max and accumulated sum (ACT), reciprocal (DVE)
  stage3: g transposed back to [q, a] via PE transpose, then
          h[a, :] = sum_q g[q, a] * a_bar[q, :] with f32r matmuls
          (lhsT = g chunks, rhs = a_bar natural), scaled by 1/Z on the
          PSUM->SBUF copy (ACT), DMA out.

All matmuls and transposes run at float32r (e8m11-ish, 1 cyc/row for the
512/256-wide matmuls, 1.5 for transposes) — 4x the fp32 matmul rate.
Rounding T/a/q/Wg to e8m11 perturbs scores by ~0.21 RMS (score std ~1024,
softmax near-one-hot); measured output rel err ~7e-3 vs the 2e-2 gate.

Phase B pipeline (per a-tile iteration i), chosen so no engine FIFO head
ever blocks a ready op:
  PE   : aT-transposes(i+1) | scores(i) | g-transposes(i-1) | stage3(i-1)
  DVE  : aT-copies(i+1), g-copies(i-1), maxes(i), combine(i), recip(i-1)
  ACT  : exp(i), h-scale(i-1)
  sync : anat(i+1) DMA (latency-critical row loads, double-buffered)
  gpsimd queue: bulk a_r fills + h stores (kept off the sync queue so they
  never delay the per-tile row loads).
"""
import os
import sys

sys.path.insert(0, "/opt/trn_rl_repo")

from contextlib import ExitStack

import numpy as np

B, S, H = 16, 2048, 1024
NCORES = 8
BPC = B // NCORES  # 2 batches per core

_cache = {}


def _build():
    import concourse.tile as tile
    from concourse import bacc, mybir
    from concourse.masks import make_identity

    F32 = mybir.dt.float32
    F32R = mybir.dt.float32r

    KC = H // 128  # 8 contraction chunks
    Q1 = 256       # stage-1 q chunk width
    AT = S // 128  # 16 a-tiles
    HC2 = H // 512  # 2 output h chunks

    nc = bacc.Bacc("TRN2", target_bir_lowering=False, debug=False,
                   num_devices=NCORES)
    q_d = nc.declare_dram_parameter("q_bar", [BPC, S, H], F32, isOutput=False)
    a_d = nc.declare_dram_parameter("a_bar", [BPC, S, H], F32, isOutput=False)
    wg_d = nc.declare_dram_parameter("Wg", [H, H], F32, isOutput=False)
    bg_d = nc.declare_dram_parameter("bg", [H], F32, isOutput=False)
    out_d = nc.declare_dram_parameter("out", [BPC, S, H], F32, isOutput=True)

    with tile.TileContext(nc) as tc, ExitStack() as ctx:
        const = ctx.enter_context(tc.tile_pool(name="const", bufs=1))
        big = ctx.enter_context(tc.tile_pool(name="big", bufs=1))
        st1 = ctx.enter_context(tc.tile_pool(name="st1", bufs=1))
        qg = ctx.enter_context(tc.tile_pool(name="qg", bufs=2))
        ld = ctx.enter_context(tc.tile_pool(name="ld", bufs=2))
        atp = ctx.enter_context(tc.tile_pool(name="atp", bufs=2))
        st2 = ctx.enter_context(tc.tile_pool(name="st2", bufs=2))
        st_ps = ctx.enter_context(tc.tile_pool(name="st_ps", bufs=1, space="PSUM"))
        tr_ps = ctx.enter_context(tc.tile_pool(name="tr_ps", bufs=2, space="PSUM"))
        h_ps = ctx.enter_context(tc.tile_pool(name="h_ps", bufs=1, space="PSUM"))

        # identity is the MOVING operand of every f32r transpose, so the
        # verifier wants its producer f32r-typed; gpsimd can't memset f32r,
        # so build it in f32 and cast-copy once.
        id32 = const.tile([128, 128], F32, tag="ident32")
        make_identity(nc, id32[:])
        idt = const.tile([128, 128], F32R, tag="ident")
        nc.vector.tensor_copy(idt[:], id32[:])
        bgt = const.tile([128, 8], F32, tag="bg")
        bg_sb = bgt[:, 0:8]                              # bg[k] at [k%128, k//128]
        nc.sync.dma_start(bg_sb, bg_d.rearrange("(ko p) -> p ko", p=128))
        wg_sb = const.tile([128, KC, H], F32, tag="wg")  # [h_in_chunk, hc, k]
        nc.sync.dma_start(wg_sb[:].bitcast(F32R),
                          wg_d.rearrange("(ho p) k -> p ho k", p=128).bitcast(F32R))

        for b in range(BPC):
            # T^T: [k within chunk, kc, q]
            T_sb = big.tile([128, KC, S], F32, tag="T")
            # a_bar natural: [q within chunk, sc, h]; consumed only by the
            # f32r stage-3 matmuls, so DMA'd with f32r-typed APs.
            a_r = big.tile([128, AT, H], F32, tag="ar")

            state = {}

            def emit_aT(i):
                anat = ld.tile([128, H], F32, tag="ld1024")
                nc.sync.dma_start(anat[:].bitcast(F32R),
                                  a_d[b, i * 128:(i + 1) * 128, :].bitcast(F32R))
                aT = atp.tile([128, KC, 128], F32, tag="aT")
                for hg in range(2):
                    pt = tr_ps.tile([128, 512], F32R, tag="tr")
                    for j in range(4):
                        kc = hg * 4 + j
                        nc.tensor.transpose(
                            pt[:, j * 128:(j + 1) * 128],
                            anat[:, kc * 128:(kc + 1) * 128].bitcast(F32R),
                            idt[:],
                        )
                    nc.vector.tensor_copy(
                        aT[:, hg * 4:(hg + 1) * 4, :].bitcast(F32R),
                        pt[:].rearrange("p (j q) -> p j q", j=4),
                    )
                state[(i, "aT")] = aT

            def emit_scores(i):
                aT = state.pop((i, "aT"))
                stt = st_ps.tile([128, 2048], F32, tag="st")
                # kc-outer: all four banks' accumulation groups start/stop
                # together, paying the PE drain once per a-tile instead of
                # once per bank.
                for kc in range(KC):
                    for qcc in range(S // 512):
                        nc.tensor.matmul(
                            stt[:, qcc * 512:(qcc + 1) * 512],
                            aT[:, kc, :].bitcast(F32R),
                            T_sb[:, kc, qcc * 512:(qcc + 1) * 512].bitcast(F32R),
                            start=(kc == 0),
                            stop=(kc == KC - 1),
                        )
                state[i] = stt

            def emit_maxes(i):
                stt = state[i]
                # softmax max over q (free axis), per bank + combine (DVE)
                stat = st2.tile([128, 8], F32, tag="stats")
                for qm in range(4):
                    nc.vector.tensor_reduce(
                        stat[:, 4 + qm:5 + qm], stt[:, qm * 512:(qm + 1) * 512],
                        axis=mybir.AxisListType.X, op=mybir.AluOpType.max,
                    )
                nc.vector.tensor_reduce(
                    stat[:, 0:1], stat[:, 4:8], axis=mybir.AxisListType.X,
                    op=mybir.AluOpType.max, negate=True,
                )
                state[(i, "stat")] = stat

            def emit_exp(i):
                stt = state.pop(i)
                stat = state.pop((i, "stat"))
                # gT shares the double-buffered qg pool with stage-1's qT
                # tiles (same 8KB/partition shape, disjoint lifetimes).
                gT = qg.tile([128, S], F32, tag="qg2k")
                # per-bank exps (same global -max bias): each frees its score
                # PSUM bank as it completes, so scores(i+1) can start before
                # the whole exp pass is done. Per-bank sums land in stat[4+qm]
                # (the bank maxes, already combined into stat[0] by now) and
                # are summed in emit_back_post.
                for qm in range(4):
                    nc.scalar.activation(
                        gT[:, qm * 512:(qm + 1) * 512].bitcast(F32R),
                        stt[:, qm * 512:(qm + 1) * 512],
                        mybir.ActivationFunctionType.Exp,
                        bias=stat[:, 0:1], scale=1.0,
                        accum_out=stat[:, 4 + qm:5 + qm],
                    )
                state[(i, "g")] = (gT, stat)

            def emit_back_pre(i):
                gT, stat = state[(i, "g")]
                g_r = st1.tile([128, AT, 128], F32R, tag="gr")
                for qgi in range(4):  # 16 transposes, batched 4 per bank
                    pt = tr_ps.tile([128, 512], F32R, tag="tr")
                    for j in range(4):
                        qc = qgi * 4 + j
                        nc.tensor.transpose(
                            pt[:, j * 128:(j + 1) * 128],
                            gT[:, qc * 128:(qc + 1) * 128].bitcast(F32R),
                            idt[:],
                        )
                    # alternate DVE/ACT casts: the PE transposes outrun a
                    # single engine's casts and starve on tr buffers.
                    dst = g_r[:, qgi * 4:(qgi + 1) * 4, :]
                    src = pt[:].rearrange("p (j q) -> p j q", j=4)
                    if qgi % 2 == 0:
                        nc.vector.tensor_copy(dst, src)
                    else:
                        nc.scalar.copy(dst, src)
                hp = h_ps.tile([128, H], F32, tag="hp")
                for hc2 in range(HC2):
                    for qq in range(AT):
                        nc.tensor.matmul(
                            hp[:, hc2 * 512:(hc2 + 1) * 512],
                            g_r[:, qq, :],
                            a_r[:, qq, hc2 * 512:(hc2 + 1) * 512].bitcast(F32R),
                            start=(qq == 0),
                            stop=(qq == AT - 1),
                        )
                state[(i, "hp")] = hp

            def emit_back_post(i):
                gT, stat = state.pop((i, "g"))
                hp = state.pop((i, "hp"))
                nc.vector.tensor_reduce(
                    stat[:, 1:2], stat[:, 4:8], axis=mybir.AxisListType.X,
                    op=mybir.AluOpType.add,
                )
                nc.vector.reciprocal(stat[:, 2:3], stat[:, 1:2])
                h_sb = st1.tile([128, H], F32, tag="h")
                nc.scalar.mul(h_sb[:], hp[:], stat[:, 2:3])
                nc.gpsimd.dma_start(out_d[b, i * 128:(i + 1) * 128, :], h_sb[:])

            # ---- stage 1: T^T = Wg^T-contraction with q^T ----
            NQC = S // Q1

            def emit_qT(qc):
                # build qT for chunk qc into a double-buffered qg tile; the
                # build for qc+1 is emitted ahead of qc's matmuls, so the
                # transposes/copies overlap the previous chunk's matmuls.
                qTt = qg.tile([128, S], F32, tag="qg2k")
                qTv = qTt[:].rearrange("p (kc q) -> p kc q", q=Q1)
                for qsc in range(Q1 // 128):
                    qnat = ld.tile([128, H], F32, tag="ld1024")
                    row0 = qc * Q1 + qsc * 128
                    nc.sync.dma_start(qnat[:].bitcast(F32R),
                                      q_d[b, row0:row0 + 128, :].bitcast(F32R))
                    for hg in range(2):  # two groups of 4 transposes per bank
                        pt = tr_ps.tile([128, 512], F32R, tag="tr")
                        for j in range(4):
                            hc = hg * 4 + j
                            nc.tensor.transpose(
                                pt[:, j * 128:(j + 1) * 128],
                                qnat[:, hc * 128:(hc + 1) * 128].bitcast(F32R),
                                idt[:],
                            )
                        # split the PSUM->SBUF copies DVE/ACT so all four
                        # land within the matmul window of the prior chunk.
                        dst = qTv[:, hg * 4:(hg + 1) * 4,
                                  qsc * 128:qsc * 128 + 128].bitcast(F32R)
                        src = pt[:].rearrange("p (j q) -> p j q", j=4)
                        if hg == 0:
                            nc.vector.tensor_copy(dst, src)
                        else:
                            nc.scalar.copy(dst, src)
                state[(qc, "qT")] = qTv

            def emit_st1_mm(qc):
                qTv = state.pop((qc, "qT"))
                # one 256-wide accumulation group per 512-elem PSUM bank:
                # start=True clears has_written for the WHOLE bank, so groups
                # must not share banks; hc-outer so the four groups start and
                # stop together (one PE drain per round). Bias-adds are
                # split DVE/ACT so the banks recycle in parallel.
                st = st_ps.tile([128, 2048], F32, tag="st")
                for rnd in range(2):
                    for hc in range(KC):
                        for kg in range(4):
                            kc = rnd * 4 + kg
                            nc.tensor.matmul(
                                st[:, kg * 512:kg * 512 + Q1],
                                wg_sb[:, hc, kc * 128:(kc + 1) * 128].bitcast(F32R),
                                qTv[:, hc, :].bitcast(F32R),
                                start=(hc == 0),
                                stop=(hc == KC - 1),
                            )
                    for kg in range(4):
                        kc = rnd * 4 + kg
                        dst = T_sb[:, kc, qc * Q1:(qc + 1) * Q1].bitcast(F32R)
                        src = st[:, kg * 512:kg * 512 + Q1]
                        bias = bg_sb[:, kc:kc + 1]
                        if kg % 2 == 0:
                            nc.vector.tensor_scalar_add(dst, src, bias)
                        else:
                            nc.scalar.add(dst, src, bias)

            emit_qT(0)
            for qc in range(NQC):  # 8 chunks of 256 q
                if qc + 1 < NQC:
                    emit_qT(qc + 1)
                if qc == NQC - 1:
                    # prefetch the first phase-B aT build into the stage-1
                    # tail so scores(0) can start right after the last
                    # stage-1 matmul.
                    emit_aT(0)
                emit_st1_mm(qc)
            emit_aT(1)

            # ---- a_r fill: bulk DMAs on the gpsimd queue ----
            for sc in range(AT):
                nc.gpsimd.dma_start(
                    a_r[:, sc, :].bitcast(F32R),
                    a_d[b, sc * 128:(sc + 1) * 128, :].bitcast(F32R),
                )

            # per-iteration emission order: DVE FIFO becomes
            #   aT-copies(i+1), g-copies(i-1), maxes(i)+combine, Zsum/recip
            # (g-copies ahead of maxes so stage3(i-1) is fed promptly); the
            # per-bank exps then free the score PSUM banks incrementally so
            # scores(i+1) never waits on the full exp pass.
            prev = None
            for i in range(AT + 1):
                if i < AT:
                    if 2 <= i + 1 < AT:
                        emit_aT(i + 1)
                    emit_scores(i)
                if prev is not None:
                    emit_back_pre(prev)
                if i < AT:
                    emit_maxes(i)
                    emit_exp(i)
                if prev is not None:
                    emit_back_post(prev)
                prev = i if i < AT else None

    nc.compile()
    return nc


def _get_nc():
    if "nc" not in _cache:
        _cache["nc"] = _build()
    return _cache["nc"]


def _run(q_bar, a_bar, Wg, bg, trace=False):
    from concourse.bass_utils import run_bass_kernel_spmd

    q_bar = np.ascontiguousarray(q_bar, dtype=np.float32)
    a_bar = np.ascontiguousarray(a_bar, dtype=np.float32)
    Wg = np.ascontiguousarray(Wg, dtype=np.float32)
    bg = np.ascontiguousarray(bg, dtype=np.float32)

    nc = _get_nc()
    in_maps = []
    for c in range(NCORES):
        in_maps.append({
            "q_bar": q_bar[c * BPC:(c + 1) * BPC],
            "a_bar": a_bar[c * BPC:(c + 1) * BPC],
            "Wg": Wg,
            "bg": bg,
        })
    res = run_bass_kernel_spmd(nc, in_maps, list(range(NCORES)), trace=trace)
    out = np.concatenate([res.results[c]["out"] for c in range(NCORES)], axis=0)
    return out, res


def kernel(q_bar, a_bar, Wg, bg):
    out, _ = _run(q_bar, a_bar, Wg, bg, trace=False)
    return out
